# revision 12
# baseline (speedup 1.0000x reference)
"""CASSI colored-aperture layer (nn_CASSI_layer_Colored) on 8 Trainium2 NeuronCores.

Reference semantics (B=4, M=N=KERN=256, L=24 bands, S=22 shots):
    h_l[s,n] = (wr*fr[l] + wg*fg[l] + wb*fb[l] + wc*fc[l]) / (wr+wg+wb+wc)
    Y[b,m,n',s] = sum_l h_l[s,n'-l] * x[b,m,n'-l,l]          (dispersion shift-sum)
    X[b,m,n,l]  = sum_s h_l[s,n] * Y[b,m,n+l,s]              (adjoint + shot sum)
    out = X / max(X)

Sharding: data-parallel over (batch b, row-half mh): 4 x 2 = 8 cores; rows m
never couple, so each core computes 128 rows of one batch independently; only
the final global max couples shards (host side, after the gather).

Per-core engine plan (partitions = 128 m-rows, free dims s-major (s, n)):
  h-build (exact, sum_c a_c = 1 trick):    h = (a0*G0l + F3l) + a1*G1l + a2*G2l
    ACT does the big scalar products, DVE the 4x tensor_scalar remainder and
    the 2x adds, Pool one fused scalar_tensor_tensor (mult+add in one op).
  Stage 1:  p = h * x_l (bcast over s);  Y += p with the first 8 shot-stripes
    accumulated by the idle PE via identity matmuls into PSUM (1 bank/stripe,
    band-0 writes the full padded 279-width to initialise the accumulation
    group), remaining 14 shots via DVE/Pool adds into SBUF fp16.
  Stage 2:  t = h * Y[:, l:l+N];  X_l = shot-sum entirely on PE (22 stripe
    matmuls accumulating into a double-buffered PSUM bank), ACT copies
    PSUM->SBUF, DMA out f32.
  h is spilled to DRAM between stages (first NCACHE bands stay in SBUF).
"""

import numpy as np

B, M, N, L, S = 4, 256, 256, 24, 22
MSH = M // 2                     # rows per core
NCORES = 8
NS, NL = N * S, N * L
NP = N + L - 1                   # 279 shifted columns
SPE = 8                          # shot-stripes accumulated on PE (1 PSUM bank each)
SSB = S - SPE                    # shot-stripes accumulated on DVE/Pool in SBUF
BANK = 512                       # PSUM bank, in f32 elements
NCACHE = 3                       # h bands kept in SBUF (rest spilled to DRAM)

# engine split knobs (stripe counts), tuned against TimelineSim.  walrus
# rejects TensorScalarPtr on Pool, so Pool only runs tensor_tensor/memset.
CSB_DVE = 9                      # of the 14 pSB stripes of C, multiplies on DVE
G_DVE = 18                       # of the 22 t-stripes, multiplies done on DVE
# bands whose h is rebuilt in stage 2 on the then-idle engines instead of
# being spilled+reloaded through the DMA (stage 2 is reload-bound)
RECOMP = (5, 8, 11, 14, 17, 20)


def _bases() -> np.ndarray:
    """(4, L) color responses paired row-wise with (wr, wg, wb, wc)."""
    wl = np.linspace(400.0, 700.0, L)

    def g(mu: float, sig: float) -> np.ndarray:
        return np.exp(-0.5 * ((wl - mu) / sig) ** 2)

    # reference: H = wr*f620 + wg*f550 + wb*f450 + wc*f500 (fr,fg,fc,fb = 620,550,500,450)
    return np.stack([g(620.0, 50.0), g(550.0, 50.0), g(450.0, 50.0), g(500.0, 50.0)])


_NC = None


def _build():
    import concourse.bacc as bacc
    import concourse.mybir as mybir
    import concourse.tile as tile

    f16, f32 = mybir.dt.float16, mybir.dt.float32
    A = mybir.AluOpType
    Copy = mybir.ActivationFunctionType.Copy
    F = _bases()
    # h = a0*G0 + a1*G1 + a2*G2 + F3  (uses sum_c a_c = 1)
    G0 = F[0] - F[3]
    G1 = F[1] - F[3]
    G2 = F[2] - F[3]
    F3 = F[3]

    nc = bacc.Bacc("TRN2", target_bir_lowering=False, debug=False, num_devices=NCORES)
    xin = nc.declare_dram_parameter("x16", [MSH, NL], f16, isOutput=False)   # (l, n)
    wins = [
        nc.declare_dram_parameter(f"a{i}", [MSH, NS], f16, isOutput=False)   # (s, n)
        for i in range(3)
    ]
    eye_in = nc.declare_dram_parameter("eye", [MSH, MSH], f16, isOutput=False)
    out = nc.declare_dram_parameter("out", [MSH, NL], f32, isOutput=True)    # (l, n)
    hcache = nc.dram_tensor("hcache", [L, MSH, NS], f16)

    with tile.TileContext(nc) as tc:
        with (
            tc.tile_pool(name="main", bufs=1) as main,
            tc.tile_pool(name="hp", bufs=3) as hp,
            tc.tile_pool(name="mp", bufs=2) as mp,
            tc.tile_pool(name="pp", bufs=2) as pp,
            tc.tile_pool(name="tp", bufs=2) as tp,
            tc.psum_pool(name="ps", bufs=1) as ps,
        ):
            w = [main.tile([MSH, NS], f16, tag=f"w{i}", name=f"w{i}t") for i in range(3)]
            xt = main.tile([MSH, NL], f16, tag="x", name="xt")
            eyet = main.tile([MSH, MSH], f16, tag="eye", name="eyet")
            Yt = main.tile([MSH, S * NP], f16, tag="Y", name="Yt")
            hkeep = main.tile([MSH, NCACHE * NS], f16, tag="hk", name="hkeep")

            for i in range(3):
                nc.sync.dma_start(w[i][:], wins[i][:])
            nc.sync.dma_start(xt[:], xin[:])
            nc.sync.dma_start(eyet[:], eye_in[:])
            nc.gpsimd.memset(Yt[:], 0.0)
            a0, a1, a2 = w[0], w[1], w[2]

            x3 = xt[:].rearrange("p (l n) -> p l n", n=N)
            Y3 = Yt[:].rearrange("p (s n) -> p s n", n=NP)

            # PSUM: stage 1 uses all 8 banks as Y shot-stripes (one per bank,
            # cols 0:NP used); stage 2 reuses banks 0/1 as the X accumulator.
            psum = ps.tile([MSH, 8 * BANK], f32, tag="ps", name="psum")
            ps3 = psum[:].rearrange("p (s b) -> p s b", b=BANK)

            # --- stage 1: Y[s, l+n] += h_l[s, n] * x[l, n] ------------------
            for l in range(L):
                g0, g1, g2, f3 = float(G0[l]), float(G1[l]), float(G2[l]), float(F3[l])
                if l < NCACHE:
                    h = hkeep[:, l * NS : (l + 1) * NS]
                else:
                    h = hp.tile([MSH, NS], f16, tag="h", name="ht")[:]
                m1 = mp.tile([MSH, NS], f16, tag="m", bufs=3, name="m1t")
                m2 = mp.tile([MSH, NS], f16, tag="m", bufs=3, name="m2t")
                nc.scalar.activation(h, a0[:], Copy, bias=f3, scale=g0)
                nc.scalar.mul(m1[:], a1[:], g1)
                nc.vector.tensor_scalar_mul(m2[:], a2[:], g2)
                nc.vector.tensor_tensor(h, h, m1[:], A.add)
                nc.vector.tensor_tensor(h, h, m2[:], A.add)
                if l >= NCACHE and l not in RECOMP:
                    nc.sync.dma_start(hcache[l], h)
                h3 = h.rearrange("p (s n) -> p s n", n=N)

                # p for the PE stripes lives in a padded (SPE, NP) tile whose
                # cols 256:279 are zero, so band-l matmuls can write width
                # NP-l (band 0 covers the whole bank row to start the group).
                pPE = pp.tile([MSH, SPE * NP], f16, tag="pe", name="pPEt")
                pPE3 = pPE[:].rearrange("p (s n) -> p s n", n=NP)
                if l < 2:  # zero the pads once per rotating buffer
                    nc.gpsimd.memset(pPE3[:, :, N:], 0.0)
                pSB = pp.tile([MSH, SSB * N], f16, tag="sb", name="pSBt")
                pSB3 = pSB[:].rearrange("p (s n) -> p s n", n=N)
                xb = x3[:, l, :].unsqueeze(1)
                nc.vector.tensor_tensor(
                    pPE3[:, :, :N], h3[:, :SPE, :], xb.broadcast_to((MSH, SPE, N)),
                    A.mult,
                )
                kd = SPE + CSB_DVE
                nc.vector.tensor_tensor(
                    pSB3[:, :CSB_DVE, :], h3[:, SPE:kd, :],
                    xb.broadcast_to((MSH, CSB_DVE, N)), A.mult,
                )
                nc.gpsimd.tensor_tensor(
                    pSB3[:, CSB_DVE:, :], h3[:, kd:, :],
                    xb.broadcast_to((MSH, SSB - CSB_DVE, N)), A.mult,
                )
                # PE: Y-psum stripe accumulation
                wdt = NP - l
                for s in range(SPE):
                    nc.tensor.matmul(
                        ps3[:, s, l : l + wdt],
                        eyet[:],
                        pPE3[:, s, 0:wdt],
                        start=(l == 0),
                        stop=(l == L - 1),
                    )
                # Pool: remaining 14 stripes accumulate into SBUF Y
                ysl = Y3[:, SPE:, l : l + N]
                nc.gpsimd.tensor_tensor(ysl, ysl, pSB3, A.add)

            # --- barrier: convert the 8 PSUM stripes into SBUF Y ------------
            nc.scalar.activation(Y3[:, :SPE, :], ps3[:, :SPE, :NP], Copy)

            # --- stage 2: X[l, n] = sum_s h_l[s, n] * Y[s, l+n] -------------
            xp = [psum[:, b * BANK : b * BANK + N] for b in range(2)]
            for l in range(L):
                if l < NCACHE:
                    h = hkeep[:, l * NS : (l + 1) * NS]
                elif l in RECOMP:
                    g0, g1, g2, f3 = (
                        float(G0[l]), float(G1[l]), float(G2[l]), float(F3[l])
                    )
                    h = hp.tile([MSH, NS], f16, tag="h", name="hrt")[:]
                    m1 = mp.tile([MSH, NS], f16, tag="m", bufs=3, name="m1rt")
                    m2 = mp.tile([MSH, NS], f16, tag="m", bufs=3, name="m2rt")
                    nc.scalar.activation(h, a0[:], Copy, bias=f3, scale=g0)
                    nc.scalar.mul(m1[:], a1[:], g1)
                    nc.vector.tensor_scalar_mul(m2[:], a2[:], g2)
                    nc.vector.tensor_tensor(h, h, m1[:], A.add)
                    # split the second add: 14 stripes on DVE, 8 on Pool
                    ksp = 14 * N
                    nc.vector.tensor_tensor(
                        h[:, :ksp], h[:, :ksp], m2[:, :ksp], A.add
                    )
                    nc.gpsimd.tensor_tensor(
                        h[:, ksp:], h[:, ksp:], m2[:, ksp:], A.add
                    )
                else:
                    h = hp.tile([MSH, NS], f16, tag="h", name="hrt")[:]
                    nc.sync.dma_start(h, hcache[l])
                h3 = h.rearrange("p (s n) -> p s n", n=N)
                t = tp.tile([MSH, NS], f16, tag="t", name="tt")
                t3 = t[:].rearrange("p (s n) -> p s n", n=N)
                nc.vector.tensor_tensor(
                    t3[:, SPE:G_DVE, :], h3[:, SPE:G_DVE, :],
                    Y3[:, SPE:G_DVE, l : l + N], A.mult,
                )
                nc.vector.tensor_tensor(
                    t3[:, :SPE, :], h3[:, :SPE, :], Y3[:, :SPE, l : l + N], A.mult
                )
                nc.gpsimd.tensor_tensor(
                    t3[:, G_DVE:, :], h3[:, G_DVE:, :], Y3[:, G_DVE:, l : l + N],
                    A.mult,
                )
                xpb = xp[l % 2]
                for s in range(S):
                    nc.tensor.matmul(
                        xpb, eyet[:], t3[:, s, :],
                        start=(s == 0), stop=(s == S - 1),
                    )
                xo = tp.tile([MSH, N], f32, tag="xo", name="xot")
                nc.scalar.activation(xo[:], xpb, Copy)
                nc.sync.dma_start(out[:, l * N : (l + 1) * N], xo[:])

    nc.compile()
    return nc


def _get_nc():
    global _NC
    if _NC is None:
        _NC = _build()
    return _NC


def _make_in_maps(x, wr, wg, wb, wc):
    x = np.asarray(x, dtype=np.float32)
    ws = [np.asarray(w, dtype=np.float32).reshape(M, M, S) for w in (wr, wg, wb, wc)]
    wt = ws[0] + ws[1] + ws[2] + ws[3]
    as_ = [w / wt for w in ws[:3]]                    # a3 never needed (sum_c a_c = 1)
    eye = np.eye(MSH, dtype=np.float16)
    in_maps = []
    for core in range(NCORES):
        b, mh = divmod(core, 2)
        rows = slice(mh * MSH, (mh + 1) * MSH)
        xs = x[b, rows].transpose(0, 2, 1)            # (MSH, L, N)
        m = {
            "x16": np.ascontiguousarray(xs).reshape(MSH, NL).astype(np.float16),
            "eye": eye,
        }
        for i, a in enumerate(as_):
            asb = a[rows].transpose(0, 2, 1)          # (MSH, S, N)
            m[f"a{i}"] = np.ascontiguousarray(asb).reshape(MSH, NS).astype(np.float16)
        in_maps.append(m)
    return in_maps


def _run_shards(in_maps):
    from concourse.bass_utils import run_bass_kernel_spmd

    nc = _get_nc()
    return run_bass_kernel_spmd(nc, in_maps, list(range(NCORES)))


def kernel(x, wr, wg, wb, wc):
    res = _run_shards(_make_in_maps(x, wr, wg, wb, wc))
    X = np.empty((B, M, N, L), dtype=np.float32)
    for core in range(NCORES):
        b, mh = divmod(core, 2)
        xo = res.results[core]["out"].reshape(MSH, L, N).transpose(0, 2, 1)
        X[b, mh * MSH : (mh + 1) * MSH] = xo
    return X / X.max()


def estimate_ns() -> float:
    """Single-core cost-model estimate of the kernel duration (ns)."""
    from concourse.timeline_sim import TimelineSim

    return TimelineSim(_get_nc()).simulate()


# revision 13
# speedup vs baseline: 1.1331x; 1.1331x over previous
"""CASSI colored-aperture layer (nn_CASSI_layer_Colored) on 8 Trainium2 NeuronCores.

Reference semantics (B=4, M=N=KERN=256, L=24 bands, S=22 shots):
    h_l[s,n] = (wr*fr[l] + wg*fg[l] + wb*fb[l] + wc*fc[l]) / (wr+wg+wb+wc)
    Y[b,m,n',s] = sum_l h_l[s,n'-l] * x[b,m,n'-l,l]          (dispersion shift-sum)
    X[b,m,n,l]  = sum_s h_l[s,n] * Y[b,m,n+l,s]              (adjoint + shot sum)
    out = X / max(X)

Sharding: data-parallel over (batch b, row-half mh): 4 x 2 = 8 cores; rows m
never couple, so each core computes 128 rows of one batch independently; only
the final global max couples shards (host side, after the gather).

Per-core engine plan (partitions = 128 m-rows, free dims s-major (s, n)):
  h-build (exact, sum_c a_c = 1 trick):    h = (a0*G0l + F3l) + a1*G1l + a2*G2l
    ACT does the big scalar products, DVE the 4x tensor_scalar remainder and
    the 2x adds, Pool one fused scalar_tensor_tensor (mult+add in one op).
  Stage 1:  p = h * x_l (bcast over s);  Y += p with the first 8 shot-stripes
    accumulated by the idle PE via identity matmuls into PSUM (1 bank/stripe,
    band-0 writes the full padded 279-width to initialise the accumulation
    group), remaining 14 shots via DVE/Pool adds into SBUF fp16.
  Stage 2:  t = h * Y[:, l:l+N];  X_l = shot-sum entirely on PE (22 stripe
    matmuls accumulating into a double-buffered PSUM bank), ACT copies
    PSUM->SBUF, DMA out f32.
  h is spilled to DRAM between stages (first NCACHE bands stay in SBUF).
"""

import numpy as np

B, M, N, L, S = 4, 256, 256, 24, 22
MSH = M // 2                     # rows per core
NCORES = 8
NS, NL = N * S, N * L
NP = N + L - 1                   # 279 shifted columns
SPE = 8                          # shot-stripes accumulated on PE (1 PSUM bank each)
SSB = S - SPE                    # shot-stripes accumulated on DVE/Pool in SBUF
BANK = 512                       # PSUM bank, in f32 elements
NCACHE = 3                       # h bands kept in SBUF (rest spilled to DRAM)

# engine split knobs (stripe counts), tuned against TimelineSim.  walrus
# rejects TensorScalarPtr on Pool, so Pool only runs tensor_tensor/memset.
CSB_DVE = 9                      # of the 14 pSB stripes of C, multiplies on DVE
G_DVE = 18                       # of the 22 t-stripes, multiplies done on DVE
# bands whose h is rebuilt in stage 2 on the then-idle engines instead of
# being spilled+reloaded through the DMA (stage 2 is reload-bound)
RECOMP = ()


def _bases() -> np.ndarray:
    """(4, L) color responses paired row-wise with (wr, wg, wb, wc)."""
    wl = np.linspace(400.0, 700.0, L)

    def g(mu: float, sig: float) -> np.ndarray:
        return np.exp(-0.5 * ((wl - mu) / sig) ** 2)

    # reference: H = wr*f620 + wg*f550 + wb*f450 + wc*f500 (fr,fg,fc,fb = 620,550,500,450)
    return np.stack([g(620.0, 50.0), g(550.0, 50.0), g(450.0, 50.0), g(500.0, 50.0)])


_NC = None


def _build():
    import concourse.bacc as bacc
    import concourse.mybir as mybir
    import concourse.tile as tile

    f16, f32 = mybir.dt.float16, mybir.dt.float32
    A = mybir.AluOpType
    Copy = mybir.ActivationFunctionType.Copy
    F = _bases()
    # h = a0*G0 + a1*G1 + a2*G2 + F3  (uses sum_c a_c = 1)
    G0 = F[0] - F[3]
    G1 = F[1] - F[3]
    G2 = F[2] - F[3]
    F3 = F[3]

    nc = bacc.Bacc("TRN2", target_bir_lowering=False, debug=False, num_devices=NCORES)
    xin = nc.declare_dram_parameter("x16", [MSH, NL], f16, isOutput=False)   # (l, n)
    wins = [
        nc.declare_dram_parameter(f"a{i}", [MSH, NS], f16, isOutput=False)   # (s, n)
        for i in range(3)
    ]
    eye_in = nc.declare_dram_parameter("eye", [MSH, MSH], f16, isOutput=False)
    out = nc.declare_dram_parameter("out", [MSH, NL], f32, isOutput=True)    # (l, n)
    hcache = nc.dram_tensor("hcache", [L, MSH, NS], f16)

    with tile.TileContext(nc) as tc:
        with (
            tc.tile_pool(name="main", bufs=1) as main,
            tc.tile_pool(name="hp", bufs=4) as hp,
            tc.tile_pool(name="mp", bufs=2) as mp,
            tc.tile_pool(name="pp", bufs=2) as pp,
            tc.tile_pool(name="tp", bufs=2) as tp,
            tc.psum_pool(name="ps", bufs=1) as ps,
        ):
            w = [main.tile([MSH, NS], f16, tag=f"w{i}", name=f"w{i}t") for i in range(3)]
            eyet = main.tile([MSH, MSH], f16, tag="eye", name="eyet")
            Yt = main.tile([MSH, S * NP], f16, tag="Y", name="Yt")
            hkeep = main.tile([MSH, NCACHE * NS], f16, tag="hk", name="hkeep")

            for i in range(3):
                nc.sync.dma_start(w[i][:], wins[i][:])
            nc.sync.dma_start(eyet[:], eye_in[:])
            nc.gpsimd.memset(Yt[:], 0.0)
            a0, a1, a2 = w[0], w[1], w[2]

            Y3 = Yt[:].rearrange("p (s n) -> p s n", n=NP)

            # PSUM: stage 1 uses all 8 banks as Y shot-stripes (one per bank,
            # cols 0:NP used); stage 2 reuses banks 0/1 as the X accumulator.
            psum = ps.tile([MSH, 8 * BANK], f32, tag="ps", name="psum")
            ps3 = psum[:].rearrange("p (s b) -> p s b", b=BANK)

            # --- stage 1: Y[s, l+n] += h_l[s, n] * x[l, n] ------------------
            for l in range(L):
                g0, g1, g2, f3 = float(G0[l]), float(G1[l]), float(G2[l]), float(F3[l])
                if l < NCACHE:
                    h = hkeep[:, l * NS : (l + 1) * NS]
                else:
                    h = hp.tile([MSH, NS], f16, tag="h", name="ht")[:]
                m1 = mp.tile([MSH, NS], f16, tag="m", bufs=3, name="m1t")
                m2 = mp.tile([MSH, NS], f16, tag="m", bufs=3, name="m2t")
                nc.scalar.activation(h, a0[:], Copy, bias=f3, scale=g0)
                nc.scalar.mul(m1[:], a1[:], g1)
                nc.vector.tensor_scalar_mul(m2[:], a2[:], g2)
                nc.vector.tensor_tensor(h, h, m1[:], A.add)
                nc.vector.tensor_tensor(h, h, m2[:], A.add)
                if l >= NCACHE and l not in RECOMP:
                    nc.sync.dma_start(hcache[l], h)
                h3 = h.rearrange("p (s n) -> p s n", n=N)

                # p for the PE stripes lives in a padded (SPE, NP) tile whose
                # cols 256:279 are zero, so band-l matmuls can write width
                # NP-l (band 0 covers the whole bank row to start the group).
                pPE = pp.tile([MSH, SPE * NP], f16, tag="pe", name="pPEt")
                pPE3 = pPE[:].rearrange("p (s n) -> p s n", n=NP)
                if l < 2:  # zero the pads once per rotating buffer
                    nc.gpsimd.memset(pPE3[:, :, N:], 0.0)
                pSB = pp.tile([MSH, SSB * N], f16, tag="sb", name="pSBt")
                pSB3 = pSB[:].rearrange("p (s n) -> p s n", n=N)
                xs = pp.tile([MSH, N], f16, tag="xs", bufs=3, name="xst")
                nc.sync.dma_start(xs[:], xin[:, l * N : (l + 1) * N])
                xb = xs[:].unsqueeze(1)
                nc.vector.tensor_tensor(
                    pPE3[:, :, :N], h3[:, :SPE, :], xb.broadcast_to((MSH, SPE, N)),
                    A.mult,
                )
                kd = SPE + CSB_DVE
                nc.vector.tensor_tensor(
                    pSB3[:, :CSB_DVE, :], h3[:, SPE:kd, :],
                    xb.broadcast_to((MSH, CSB_DVE, N)), A.mult,
                )
                nc.gpsimd.tensor_tensor(
                    pSB3[:, CSB_DVE:, :], h3[:, kd:, :],
                    xb.broadcast_to((MSH, SSB - CSB_DVE, N)), A.mult,
                )
                # PE: Y-psum stripe accumulation
                wdt = NP - l
                for s in range(SPE):
                    nc.tensor.matmul(
                        ps3[:, s, l : l + wdt],
                        eyet[:],
                        pPE3[:, s, 0:wdt],
                        start=(l == 0),
                        stop=(l == L - 1),
                    )
                # Pool: remaining 14 stripes accumulate into SBUF Y
                ysl = Y3[:, SPE:, l : l + N]
                nc.gpsimd.tensor_tensor(ysl, ysl, pSB3, A.add)

            # --- barrier: convert the 8 PSUM stripes into SBUF Y ------------
            nc.scalar.activation(Y3[:, :SPE, :], ps3[:, :SPE, :NP], Copy)

            # --- stage 2: X[l, n] = sum_s h_l[s, n] * Y[s, l+n] -------------
            xp = [psum[:, b * BANK : b * BANK + N] for b in range(2)]
            for l in range(L):
                if l < NCACHE:
                    h = hkeep[:, l * NS : (l + 1) * NS]
                elif l in RECOMP:
                    g0, g1, g2, f3 = (
                        float(G0[l]), float(G1[l]), float(G2[l]), float(F3[l])
                    )
                    h = hp.tile([MSH, NS], f16, tag="h", name="hrt")[:]
                    m1 = mp.tile([MSH, NS], f16, tag="m", bufs=3, name="m1rt")
                    m2 = mp.tile([MSH, NS], f16, tag="m", bufs=3, name="m2rt")
                    nc.scalar.activation(h, a0[:], Copy, bias=f3, scale=g0)
                    nc.scalar.mul(m1[:], a1[:], g1)
                    nc.vector.tensor_scalar_mul(m2[:], a2[:], g2)
                    nc.vector.tensor_tensor(h, h, m1[:], A.add)
                    # split the second add: 14 stripes on DVE, 8 on Pool
                    ksp = 14 * N
                    nc.vector.tensor_tensor(
                        h[:, :ksp], h[:, :ksp], m2[:, :ksp], A.add
                    )
                    nc.gpsimd.tensor_tensor(
                        h[:, ksp:], h[:, ksp:], m2[:, ksp:], A.add
                    )
                else:
                    h = hp.tile([MSH, NS], f16, tag="h", name="hrt")[:]
                    nc.sync.dma_start(h, hcache[l])
                h3 = h.rearrange("p (s n) -> p s n", n=N)
                t = tp.tile([MSH, NS], f16, tag="t", name="tt")
                t3 = t[:].rearrange("p (s n) -> p s n", n=N)
                nc.vector.tensor_tensor(
                    t3[:, SPE:G_DVE, :], h3[:, SPE:G_DVE, :],
                    Y3[:, SPE:G_DVE, l : l + N], A.mult,
                )
                nc.vector.tensor_tensor(
                    t3[:, :SPE, :], h3[:, :SPE, :], Y3[:, :SPE, l : l + N], A.mult
                )
                nc.gpsimd.tensor_tensor(
                    t3[:, G_DVE:, :], h3[:, G_DVE:, :], Y3[:, G_DVE:, l : l + N],
                    A.mult,
                )
                xpb = xp[l % 2]
                for s in range(S):
                    nc.tensor.matmul(
                        xpb, eyet[:], t3[:, s, :],
                        start=(s == 0), stop=(s == S - 1),
                    )
                xo = tp.tile([MSH, N], f32, tag="xo", name="xot")
                nc.scalar.activation(xo[:], xpb, Copy)
                nc.sync.dma_start(out[:, l * N : (l + 1) * N], xo[:])

    nc.compile()
    return nc


def _get_nc():
    global _NC
    if _NC is None:
        _NC = _build()
    return _NC


def _make_in_maps(x, wr, wg, wb, wc):
    x = np.asarray(x, dtype=np.float32)
    ws = [np.asarray(w, dtype=np.float32).reshape(M, M, S) for w in (wr, wg, wb, wc)]
    wt = ws[0] + ws[1] + ws[2] + ws[3]
    as_ = [w / wt for w in ws[:3]]                    # a3 never needed (sum_c a_c = 1)
    eye = np.eye(MSH, dtype=np.float16)
    in_maps = []
    for core in range(NCORES):
        b, mh = divmod(core, 2)
        rows = slice(mh * MSH, (mh + 1) * MSH)
        xs = x[b, rows].transpose(0, 2, 1)            # (MSH, L, N)
        m = {
            "x16": np.ascontiguousarray(xs).reshape(MSH, NL).astype(np.float16),
            "eye": eye,
        }
        for i, a in enumerate(as_):
            asb = a[rows].transpose(0, 2, 1)          # (MSH, S, N)
            m[f"a{i}"] = np.ascontiguousarray(asb).reshape(MSH, NS).astype(np.float16)
        in_maps.append(m)
    return in_maps


def _run_shards(in_maps):
    from concourse.bass_utils import run_bass_kernel_spmd

    nc = _get_nc()
    return run_bass_kernel_spmd(nc, in_maps, list(range(NCORES)))


def kernel(x, wr, wg, wb, wc):
    res = _run_shards(_make_in_maps(x, wr, wg, wb, wc))
    X = np.empty((B, M, N, L), dtype=np.float32)
    for core in range(NCORES):
        b, mh = divmod(core, 2)
        xo = res.results[core]["out"].reshape(MSH, L, N).transpose(0, 2, 1)
        X[b, mh * MSH : (mh + 1) * MSH] = xo
    return X / X.max()


def estimate_ns() -> float:
    """Single-core cost-model estimate of the kernel duration (ns)."""
    from concourse.timeline_sim import TimelineSim

    return TimelineSim(_get_nc()).simulate()


# revision 15
# speedup vs baseline: 1.1544x; 1.0187x over previous
"""CASSI colored-aperture layer (nn_CASSI_layer_Colored) on 8 Trainium2 NeuronCores.

Reference semantics (B=4, M=N=KERN=256, L=24 bands, S=22 shots):
    h_l[s,n] = (wr*fr[l] + wg*fg[l] + wb*fb[l] + wc*fc[l]) / (wr+wg+wb+wc)
    Y[b,m,n',s] = sum_l h_l[s,n'-l] * x[b,m,n'-l,l]          (dispersion shift-sum)
    X[b,m,n,l]  = sum_s h_l[s,n] * Y[b,m,n+l,s]              (adjoint + shot sum)
    out = X / max(X)

Sharding: data-parallel over (batch b, row-half mh): 4 x 2 = 8 cores; rows m
never couple, so each core computes 128 rows of one batch independently; only
the final global max couples shards (host side, after the gather).

Per-core engine plan (partitions = 128 m-rows, free dims s-major (s, n)):
  h-build (exact, sum_c a_c = 1 trick):    h = (a0*G0l + F3l) + a1*G1l + a2*G2l
    ACT does the big scalar products, DVE the 4x tensor_scalar remainder and
    the 2x adds, Pool one fused scalar_tensor_tensor (mult+add in one op).
  Stage 1:  p = h * x_l (bcast over s);  Y += p with the first 8 shot-stripes
    accumulated by the idle PE via identity matmuls into PSUM (1 bank/stripe,
    band-0 writes the full padded 279-width to initialise the accumulation
    group), remaining 14 shots via DVE/Pool adds into SBUF fp16.
  Stage 2:  t = h * Y[:, l:l+N];  X_l = shot-sum entirely on PE (22 stripe
    matmuls accumulating into a double-buffered PSUM bank), ACT copies
    PSUM->SBUF, DMA out f32.
  h is spilled to DRAM between stages (first NCACHE bands stay in SBUF).
"""

import numpy as np

B, M, N, L, S = 4, 256, 256, 24, 22
MSH = M // 2                     # rows per core
NCORES = 8
NS, NL = N * S, N * L
NP = N + L - 1                   # 279 shifted columns
SPE = 8                          # shot-stripes accumulated on PE (1 PSUM bank each)
SSB = S - SPE                    # shot-stripes accumulated on DVE/Pool in SBUF
BANK = 512                       # PSUM bank, in f32 elements
NCACHE = 3                       # h bands kept in SBUF (rest spilled to DRAM)

# engine split knobs (stripe counts), tuned against TimelineSim.  walrus
# rejects TensorScalarPtr on Pool, so Pool only runs tensor_tensor/memset.
CSB_DVE = 9                      # of the 14 pSB stripes of C, multiplies on DVE
G_DVE = 18                       # of the 22 t-stripes, multiplies done on DVE
# bands whose h is rebuilt in stage 2 on the then-idle engines instead of
# being spilled+reloaded through the DMA (stage 2 is reload-bound)
RECOMP = ()


def _bases() -> np.ndarray:
    """(4, L) color responses paired row-wise with (wr, wg, wb, wc)."""
    wl = np.linspace(400.0, 700.0, L)

    def g(mu: float, sig: float) -> np.ndarray:
        return np.exp(-0.5 * ((wl - mu) / sig) ** 2)

    # reference: H = wr*f620 + wg*f550 + wb*f450 + wc*f500 (fr,fg,fc,fb = 620,550,500,450)
    return np.stack([g(620.0, 50.0), g(550.0, 50.0), g(450.0, 50.0), g(500.0, 50.0)])


_NC = None


def _build():
    import concourse.bacc as bacc
    import concourse.mybir as mybir
    import concourse.tile as tile

    f16, f32 = mybir.dt.float16, mybir.dt.float32
    A = mybir.AluOpType
    Copy = mybir.ActivationFunctionType.Copy
    F = _bases()
    # h = a0*G0 + a1*G1 + a2*G2 + F3  (uses sum_c a_c = 1)
    G0 = F[0] - F[3]
    G1 = F[1] - F[3]
    G2 = F[2] - F[3]
    F3 = F[3]

    nc = bacc.Bacc("TRN2", target_bir_lowering=False, debug=False, num_devices=NCORES)
    xin = nc.declare_dram_parameter("x16", [MSH, NL], f16, isOutput=False)   # (l, n)
    wins = [
        nc.declare_dram_parameter(f"a{i}", [MSH, NS], f16, isOutput=False)   # (s, n)
        for i in range(3)
    ]
    eye_in = nc.declare_dram_parameter("eye", [MSH, MSH], f16, isOutput=False)
    out = nc.declare_dram_parameter("out", [MSH, NL], f16, isOutput=True)    # (l, n)
    hcache = nc.dram_tensor("hcache", [L, MSH, NS], f16)

    with tile.TileContext(nc) as tc:
        with (
            tc.tile_pool(name="main", bufs=1) as main,
            tc.tile_pool(name="hp", bufs=4) as hp,
            tc.tile_pool(name="mp", bufs=2) as mp,
            tc.tile_pool(name="pp", bufs=2) as pp,
            tc.tile_pool(name="tp", bufs=2) as tp,
            tc.psum_pool(name="ps", bufs=1) as ps,
        ):
            w = [main.tile([MSH, NS], f16, tag=f"w{i}", name=f"w{i}t") for i in range(3)]
            eyet = main.tile([MSH, MSH], f16, tag="eye", name="eyet")
            Yt = main.tile([MSH, S * NP], f16, tag="Y", name="Yt")
            hkeep = main.tile([MSH, NCACHE * NS], f16, tag="hk", name="hkeep")

            for i in range(3):
                nc.sync.dma_start(w[i][:], wins[i][:])
            nc.sync.dma_start(eyet[:], eye_in[:])
            nc.gpsimd.memset(Yt[:], 0.0)
            a0, a1, a2 = w[0], w[1], w[2]

            Y3 = Yt[:].rearrange("p (s n) -> p s n", n=NP)

            # PSUM: stage 1 uses all 8 banks as Y shot-stripes (one per bank,
            # cols 0:NP used); stage 2 reuses banks 0/1 as the X accumulator.
            psum = ps.tile([MSH, 8 * BANK], f32, tag="ps", name="psum")
            ps3 = psum[:].rearrange("p (s b) -> p s b", b=BANK)

            # --- stage 1: Y[s, l+n] += h_l[s, n] * x[l, n] ------------------
            hlive = {}                     # band -> live hp tile at stage-1 end
            for l in range(L):
                g0, g1, g2, f3 = float(G0[l]), float(G1[l]), float(G2[l]), float(F3[l])
                if l < NCACHE:
                    h = hkeep[:, l * NS : (l + 1) * NS]
                else:
                    h = hp.tile([MSH, NS], f16, tag="h", name="ht")[:]
                    hlive[l] = h
                m1 = mp.tile([MSH, NS], f16, tag="m", bufs=3, name="m1t")
                m2 = mp.tile([MSH, NS], f16, tag="m", bufs=3, name="m2t")
                nc.scalar.activation(h, a0[:], Copy, bias=f3, scale=g0)
                nc.scalar.mul(m1[:], a1[:], g1)
                nc.vector.tensor_scalar_mul(m2[:], a2[:], g2)
                nc.vector.tensor_tensor(h, h, m1[:], A.add)
                nc.vector.tensor_tensor(h, h, m2[:], A.add)
                if NCACHE <= l < L - 4 and l not in RECOMP:
                    nc.sync.dma_start(hcache[l], h)
                h3 = h.rearrange("p (s n) -> p s n", n=N)

                # p for the PE stripes lives in a padded (SPE, NP) tile whose
                # cols 256:279 are zero, so band-l matmuls can write width
                # NP-l (band 0 covers the whole bank row to start the group).
                pPE = pp.tile([MSH, SPE * NP], f16, tag="pe", name="pPEt")
                pPE3 = pPE[:].rearrange("p (s n) -> p s n", n=NP)
                if l < 2:  # zero the pads once per rotating buffer
                    nc.gpsimd.memset(pPE3[:, :, N:], 0.0)
                pSB = pp.tile([MSH, SSB * N], f16, tag="sb", name="pSBt")
                pSB3 = pSB[:].rearrange("p (s n) -> p s n", n=N)
                xs = pp.tile([MSH, N], f16, tag="xs", bufs=3, name="xst")
                nc.sync.dma_start(xs[:], xin[:, l * N : (l + 1) * N])
                xb = xs[:].unsqueeze(1)
                nc.vector.tensor_tensor(
                    pPE3[:, :, :N], h3[:, :SPE, :], xb.broadcast_to((MSH, SPE, N)),
                    A.mult,
                )
                kd = SPE + CSB_DVE
                nc.vector.tensor_tensor(
                    pSB3[:, :CSB_DVE, :], h3[:, SPE:kd, :],
                    xb.broadcast_to((MSH, CSB_DVE, N)), A.mult,
                )
                nc.gpsimd.tensor_tensor(
                    pSB3[:, CSB_DVE:, :], h3[:, kd:, :],
                    xb.broadcast_to((MSH, SSB - CSB_DVE, N)), A.mult,
                )
                # PE: Y-psum stripe accumulation
                wdt = NP - l
                for s in range(SPE):
                    nc.tensor.matmul(
                        ps3[:, s, l : l + wdt],
                        eyet[:],
                        pPE3[:, s, 0:wdt],
                        start=(l == 0),
                        stop=(l == L - 1),
                    )
                # Pool: remaining 14 stripes accumulate into SBUF Y
                ysl = Y3[:, SPE:, l : l + N]
                nc.gpsimd.tensor_tensor(ysl, ysl, pSB3, A.add)

            # --- barrier: convert the 8 PSUM stripes into SBUF Y ------------
            nc.scalar.activation(Y3[:, :SPE, :], ps3[:, :SPE, :NP], Copy)

            # --- stage 2: X[l, n] = sum_s h_l[s, n] * Y[s, l+n] -------------
            xp = [psum[:, b * BANK : b * BANK + N] for b in range(2)]
            order = [23, 22, 21, 20] + list(range(NCACHE)) + list(range(NCACHE, L - 4))
            for it, l in enumerate(order):
                if l >= L - 4:
                    h = hlive[l]
                elif l < NCACHE:
                    h = hkeep[:, l * NS : (l + 1) * NS]
                elif l in RECOMP:
                    g0, g1, g2, f3 = (
                        float(G0[l]), float(G1[l]), float(G2[l]), float(F3[l])
                    )
                    h = hp.tile([MSH, NS], f16, tag="h", name="hrt")[:]
                    m1 = mp.tile([MSH, NS], f16, tag="m", bufs=3, name="m1rt")
                    m2 = mp.tile([MSH, NS], f16, tag="m", bufs=3, name="m2rt")
                    nc.scalar.activation(h, a0[:], Copy, bias=f3, scale=g0)
                    nc.scalar.mul(m1[:], a1[:], g1)
                    nc.vector.tensor_scalar_mul(m2[:], a2[:], g2)
                    nc.vector.tensor_tensor(h, h, m1[:], A.add)
                    # split the second add: 14 stripes on DVE, 8 on Pool
                    ksp = 14 * N
                    nc.vector.tensor_tensor(
                        h[:, :ksp], h[:, :ksp], m2[:, :ksp], A.add
                    )
                    nc.gpsimd.tensor_tensor(
                        h[:, ksp:], h[:, ksp:], m2[:, ksp:], A.add
                    )
                else:
                    h = hp.tile([MSH, NS], f16, tag="h", name="hrt")[:]
                    nc.sync.dma_start(h, hcache[l])
                h3 = h.rearrange("p (s n) -> p s n", n=N)
                t = tp.tile([MSH, NS], f16, tag="t", name="tt")
                t3 = t[:].rearrange("p (s n) -> p s n", n=N)
                nc.vector.tensor_tensor(
                    t3[:, SPE:G_DVE, :], h3[:, SPE:G_DVE, :],
                    Y3[:, SPE:G_DVE, l : l + N], A.mult,
                )
                nc.vector.tensor_tensor(
                    t3[:, :SPE, :], h3[:, :SPE, :], Y3[:, :SPE, l : l + N], A.mult
                )
                nc.gpsimd.tensor_tensor(
                    t3[:, G_DVE:, :], h3[:, G_DVE:, :], Y3[:, G_DVE:, l : l + N],
                    A.mult,
                )
                xpb = xp[it % 2]
                for s in range(S):
                    nc.tensor.matmul(
                        xpb, eyet[:], t3[:, s, :],
                        start=(s == 0), stop=(s == S - 1),
                    )
                xo = tp.tile([MSH, N], f16, tag="xo", name="xot")
                nc.scalar.activation(xo[:], xpb, Copy)
                nc.sync.dma_start(out[:, l * N : (l + 1) * N], xo[:])

    nc.compile()
    return nc


def _get_nc():
    global _NC
    if _NC is None:
        _NC = _build()
    return _NC


def _make_in_maps(x, wr, wg, wb, wc):
    x = np.asarray(x, dtype=np.float32)
    ws = [np.asarray(w, dtype=np.float32).reshape(M, M, S) for w in (wr, wg, wb, wc)]
    wt = ws[0] + ws[1] + ws[2] + ws[3]
    as_ = [w / wt for w in ws[:3]]                    # a3 never needed (sum_c a_c = 1)
    eye = np.eye(MSH, dtype=np.float16)
    in_maps = []
    for core in range(NCORES):
        b, mh = divmod(core, 2)
        rows = slice(mh * MSH, (mh + 1) * MSH)
        xs = x[b, rows].transpose(0, 2, 1)            # (MSH, L, N)
        m = {
            "x16": np.ascontiguousarray(xs).reshape(MSH, NL).astype(np.float16),
            "eye": eye,
        }
        for i, a in enumerate(as_):
            asb = a[rows].transpose(0, 2, 1)          # (MSH, S, N)
            m[f"a{i}"] = np.ascontiguousarray(asb).reshape(MSH, NS).astype(np.float16)
        in_maps.append(m)
    return in_maps


def _run_shards(in_maps):
    from concourse.bass_utils import run_bass_kernel_spmd

    nc = _get_nc()
    return run_bass_kernel_spmd(nc, in_maps, list(range(NCORES)))


def kernel(x, wr, wg, wb, wc):
    res = _run_shards(_make_in_maps(x, wr, wg, wb, wc))
    X = np.empty((B, M, N, L), dtype=np.float32)
    for core in range(NCORES):
        b, mh = divmod(core, 2)
        xo = res.results[core]["out"].reshape(MSH, L, N).transpose(0, 2, 1)
        X[b, mh * MSH : (mh + 1) * MSH] = xo
    return X / X.max()


def estimate_ns() -> float:
    """Single-core cost-model estimate of the kernel duration (ns)."""
    from concourse.timeline_sim import TimelineSim

    return TimelineSim(_get_nc()).simulate()


# revision 16
# speedup vs baseline: 1.1880x; 1.0292x over previous
"""CASSI colored-aperture layer (nn_CASSI_layer_Colored) on 8 Trainium2 NeuronCores.

Reference semantics (B=4, M=N=KERN=256, L=24 bands, S=22 shots):
    h_l[s,n] = (wr*fr[l] + wg*fg[l] + wb*fb[l] + wc*fc[l]) / (wr+wg+wb+wc)
    Y[b,m,n',s] = sum_l h_l[s,n'-l] * x[b,m,n'-l,l]          (dispersion shift-sum)
    X[b,m,n,l]  = sum_s h_l[s,n] * Y[b,m,n+l,s]              (adjoint + shot sum)
    out = X / max(X)

Sharding: data-parallel over (batch b, row-half mh): 4 x 2 = 8 cores; rows m
never couple, so each core computes 128 rows of one batch independently; only
the final global max couples shards (host side, after the gather).

Per-core engine plan (partitions = 128 m-rows, free dims s-major (s, n)):
  h-build (exact, sum_c a_c = 1 trick):    h = (a0*G0l + F3l) + a1*G1l + a2*G2l
    ACT does the big scalar products, DVE the 4x tensor_scalar remainder and
    the 2x adds, Pool one fused scalar_tensor_tensor (mult+add in one op).
  Stage 1:  p = h * x_l (bcast over s);  Y += p with the first 8 shot-stripes
    accumulated by the idle PE via identity matmuls into PSUM (1 bank/stripe,
    band-0 writes the full padded 279-width to initialise the accumulation
    group), remaining 14 shots via DVE/Pool adds into SBUF fp16.
  Stage 2:  t = h * Y[:, l:l+N];  X_l = shot-sum entirely on PE (22 stripe
    matmuls accumulating into a double-buffered PSUM bank), ACT copies
    PSUM->SBUF, DMA out f32.
  h is spilled to DRAM between stages (first NCACHE bands stay in SBUF).
"""

import numpy as np

B, M, N, L, S = 4, 256, 256, 24, 22
MSH = M // 2                     # rows per core
NCORES = 8
NS, NL = N * S, N * L
NP = N + L - 1                   # 279 shifted columns
SPE = 8                          # shot-stripes accumulated on PE (1 PSUM bank each)
SSB = S - SPE                    # shot-stripes accumulated on DVE/Pool in SBUF
BANK = 512                       # PSUM bank, in f32 elements
NCACHE = 3                       # h bands kept in SBUF (rest spilled to DRAM)

# engine split knobs (stripe counts), tuned against TimelineSim.  walrus
# rejects TensorScalarPtr on Pool, so Pool only runs tensor_tensor/memset.
CSB_DVE = 9                      # of the 14 pSB stripes of C, multiplies on DVE
G_DVE = 18                       # of the 22 t-stripes, multiplies done on DVE
# bands whose h is rebuilt in stage 2 on the then-idle engines instead of
# being spilled+reloaded through the DMA (stage 2 is reload-bound)
RECOMP = ()


def _bases() -> np.ndarray:
    """(4, L) color responses paired row-wise with (wr, wg, wb, wc)."""
    wl = np.linspace(400.0, 700.0, L)

    def g(mu: float, sig: float) -> np.ndarray:
        return np.exp(-0.5 * ((wl - mu) / sig) ** 2)

    # reference: H = wr*f620 + wg*f550 + wb*f450 + wc*f500 (fr,fg,fc,fb = 620,550,500,450)
    return np.stack([g(620.0, 50.0), g(550.0, 50.0), g(450.0, 50.0), g(500.0, 50.0)])


_NC = None


def _build():
    import concourse.bacc as bacc
    import concourse.mybir as mybir
    import concourse.tile as tile

    f16, f32 = mybir.dt.float16, mybir.dt.float32
    A = mybir.AluOpType
    Copy = mybir.ActivationFunctionType.Copy
    F = _bases()
    # h = a0*G0 + a1*G1 + a2*G2 + F3  (uses sum_c a_c = 1)
    G0 = F[0] - F[3]
    G1 = F[1] - F[3]
    G2 = F[2] - F[3]
    F3 = F[3]

    nc = bacc.Bacc("TRN2", target_bir_lowering=False, debug=False, num_devices=NCORES)
    xin = nc.declare_dram_parameter("x16", [MSH, NL], f16, isOutput=False)   # (l, n)
    wins = [
        nc.declare_dram_parameter(f"a{i}", [MSH, NS], f16, isOutput=False)   # (s, n)
        for i in range(3)
    ]
    eye_in = nc.declare_dram_parameter("eye", [MSH, MSH], f16, isOutput=False)
    out = nc.declare_dram_parameter("out", [MSH, NL], f16, isOutput=True)    # (l, n)
    hcache = nc.dram_tensor("hcache", [L, MSH, NS], f16)

    with tile.TileContext(nc) as tc:
        with (
            tc.tile_pool(name="main", bufs=1) as main,
            tc.tile_pool(name="hp", bufs=4) as hp,
            tc.tile_pool(name="mp", bufs=2) as mp,
            tc.tile_pool(name="pp", bufs=2) as pp,
            tc.tile_pool(name="tp", bufs=2) as tp,
            tc.psum_pool(name="ps", bufs=1) as ps,
        ):
            w = [main.tile([MSH, NS], f16, tag=f"w{i}", name=f"w{i}t") for i in range(3)]
            eyet = main.tile([MSH, MSH], f16, tag="eye", name="eyet")
            Yt = main.tile([MSH, S * NP], f16, tag="Y", name="Yt")
            hkeep = main.tile([MSH, NCACHE * NS], f16, tag="hk", name="hkeep")

            for i in range(3):
                nc.sync.dma_start(w[i][:], wins[i][:])
            nc.sync.dma_start(eyet[:], eye_in[:])
            nc.gpsimd.memset(Yt[:], 0.0)
            a0, a1, a2 = w[0], w[1], w[2]

            Y3 = Yt[:].rearrange("p (s n) -> p s n", n=NP)

            # PSUM: stage 1 uses all 8 banks as Y shot-stripes (one per bank,
            # cols 0:NP used); stage 2 reuses banks 0/1 as the X accumulator.
            psum = ps.tile([MSH, 8 * BANK], f32, tag="ps", name="psum")
            ps3 = psum[:].rearrange("p (s b) -> p s b", b=BANK)

            # --- stage 1: Y[s, l+n] += h_l[s, n] * x[l, n] ------------------
            hlive = {}                     # band -> live hp tile at stage-1 end
            for l in range(L):
                g0, g1, g2, f3 = float(G0[l]), float(G1[l]), float(G2[l]), float(F3[l])
                if l < NCACHE:
                    h = hkeep[:, l * NS : (l + 1) * NS]
                else:
                    h = hp.tile([MSH, NS], f16, tag="h", name="ht")[:]
                    hlive[l] = h
                m1 = mp.tile([MSH, NS], f16, tag="m", bufs=3, name="m1t")
                m2 = mp.tile([MSH, NS], f16, tag="m", bufs=3, name="m2t")
                nc.scalar.activation(h, a0[:], Copy, bias=f3, scale=g0)
                nc.scalar.mul(m1[:], a1[:], g1)
                nc.vector.tensor_scalar_mul(m2[:], a2[:], g2)
                nc.vector.tensor_tensor(h, h, m1[:], A.add)
                nc.vector.tensor_tensor(h, h, m2[:], A.add)
                if NCACHE <= l < L - 4 and l not in RECOMP:
                    nc.sync.dma_start(hcache[l], h)
                h3 = h.rearrange("p (s n) -> p s n", n=N)

                # p for the PE stripes lives in a padded (SPE, NP) tile whose
                # cols 256:279 are zero, so band-l matmuls can write width
                # NP-l (band 0 covers the whole bank row to start the group).
                pPE = pp.tile([MSH, SPE * NP], f16, tag="pe", name="pPEt")
                pPE3 = pPE[:].rearrange("p (s n) -> p s n", n=NP)
                if l < 2:  # zero the pads once per rotating buffer
                    nc.gpsimd.memset(pPE3[:, :, N:], 0.0)
                pSB = pp.tile([MSH, SSB * N], f16, tag="sb", name="pSBt")
                pSB3 = pSB[:].rearrange("p (s n) -> p s n", n=N)
                xs = pp.tile([MSH, N], f16, tag="xs", bufs=3, name="xst")
                nc.sync.dma_start(xs[:], xin[:, l * N : (l + 1) * N])
                xb = xs[:].unsqueeze(1)
                nc.vector.tensor_tensor(
                    pPE3[:, :, :N], h3[:, :SPE, :], xb.broadcast_to((MSH, SPE, N)),
                    A.mult,
                )
                kd = SPE + CSB_DVE
                nc.vector.tensor_tensor(
                    pSB3[:, :CSB_DVE, :], h3[:, SPE:kd, :],
                    xb.broadcast_to((MSH, CSB_DVE, N)), A.mult,
                )
                nc.gpsimd.tensor_tensor(
                    pSB3[:, CSB_DVE:, :], h3[:, kd:, :],
                    xb.broadcast_to((MSH, SSB - CSB_DVE, N)), A.mult,
                )
                # PE: Y-psum stripe accumulation
                wdt = NP - l
                for s in range(SPE):
                    nc.tensor.matmul(
                        ps3[:, s, l : l + wdt],
                        eyet[:],
                        pPE3[:, s, 0:wdt],
                        start=(l == 0),
                        stop=(l == L - 1),
                    )
                # Pool: remaining 14 stripes accumulate into SBUF Y
                ysl = Y3[:, SPE:, l : l + N]
                nc.gpsimd.tensor_tensor(ysl, ysl, pSB3, A.add)

            # --- barrier: convert the 8 PSUM stripes into SBUF Y ------------
            nc.scalar.activation(Y3[:, :SPE, :], ps3[:, :SPE, :NP], Copy)

            # --- stage 2: X[l, n] = sum_s h_l[s, n] * Y[s, l+n] -------------
            xp = [psum[:, b * BANK : b * BANK + N] for b in range(2)]
            order = [23, 22, 21, 20] + list(range(NCACHE)) + list(range(NCACHE, L - 4))
            for it, l in enumerate(order):
                if l >= L - 4:
                    h = hlive[l]
                elif l < NCACHE:
                    h = hkeep[:, l * NS : (l + 1) * NS]
                elif l in RECOMP:
                    g0, g1, g2, f3 = (
                        float(G0[l]), float(G1[l]), float(G2[l]), float(F3[l])
                    )
                    h = hp.tile([MSH, NS], f16, tag="h", name="hrt")[:]
                    m1 = mp.tile([MSH, NS], f16, tag="m", bufs=3, name="m1rt")
                    m2 = mp.tile([MSH, NS], f16, tag="m", bufs=3, name="m2rt")
                    nc.scalar.activation(h, a0[:], Copy, bias=f3, scale=g0)
                    nc.scalar.mul(m1[:], a1[:], g1)
                    nc.vector.tensor_scalar_mul(m2[:], a2[:], g2)
                    nc.vector.tensor_tensor(h, h, m1[:], A.add)
                    # split the second add: 14 stripes on DVE, 8 on Pool
                    ksp = 14 * N
                    nc.vector.tensor_tensor(
                        h[:, :ksp], h[:, :ksp], m2[:, :ksp], A.add
                    )
                    nc.gpsimd.tensor_tensor(
                        h[:, ksp:], h[:, ksp:], m2[:, ksp:], A.add
                    )
                else:
                    if it % 2 == 0:
                        h = mp.tile([MSH, NS], f16, tag="m", bufs=3, name="hrmt")[:]
                    else:
                        h = hp.tile([MSH, NS], f16, tag="h", name="hrt")[:]
                    nc.sync.dma_start(h, hcache[l])
                h3 = h.rearrange("p (s n) -> p s n", n=N)
                t = tp.tile([MSH, NS], f16, tag="t", name="tt")
                t3 = t[:].rearrange("p (s n) -> p s n", n=N)
                nc.vector.tensor_tensor(
                    t3[:, SPE:G_DVE, :], h3[:, SPE:G_DVE, :],
                    Y3[:, SPE:G_DVE, l : l + N], A.mult,
                )
                nc.vector.tensor_tensor(
                    t3[:, :SPE, :], h3[:, :SPE, :], Y3[:, :SPE, l : l + N], A.mult
                )
                nc.gpsimd.tensor_tensor(
                    t3[:, G_DVE:, :], h3[:, G_DVE:, :], Y3[:, G_DVE:, l : l + N],
                    A.mult,
                )
                xpb = xp[it % 2]
                for s in range(S):
                    nc.tensor.matmul(
                        xpb, eyet[:], t3[:, s, :],
                        start=(s == 0), stop=(s == S - 1),
                    )
                xo = tp.tile([MSH, N], f16, tag="xo", name="xot")
                nc.scalar.activation(xo[:], xpb, Copy)
                nc.sync.dma_start(out[:, l * N : (l + 1) * N], xo[:])

    nc.compile()
    return nc


def _get_nc():
    global _NC
    if _NC is None:
        _NC = _build()
    return _NC


def _make_in_maps(x, wr, wg, wb, wc):
    x = np.asarray(x, dtype=np.float32)
    ws = [np.asarray(w, dtype=np.float32).reshape(M, M, S) for w in (wr, wg, wb, wc)]
    wt = ws[0] + ws[1] + ws[2] + ws[3]
    as_ = [w / wt for w in ws[:3]]                    # a3 never needed (sum_c a_c = 1)
    eye = np.eye(MSH, dtype=np.float16)
    in_maps = []
    for core in range(NCORES):
        b, mh = divmod(core, 2)
        rows = slice(mh * MSH, (mh + 1) * MSH)
        xs = x[b, rows].transpose(0, 2, 1)            # (MSH, L, N)
        m = {
            "x16": np.ascontiguousarray(xs).reshape(MSH, NL).astype(np.float16),
            "eye": eye,
        }
        for i, a in enumerate(as_):
            asb = a[rows].transpose(0, 2, 1)          # (MSH, S, N)
            m[f"a{i}"] = np.ascontiguousarray(asb).reshape(MSH, NS).astype(np.float16)
        in_maps.append(m)
    return in_maps


def _run_shards(in_maps):
    from concourse.bass_utils import run_bass_kernel_spmd

    nc = _get_nc()
    return run_bass_kernel_spmd(nc, in_maps, list(range(NCORES)))


def kernel(x, wr, wg, wb, wc):
    res = _run_shards(_make_in_maps(x, wr, wg, wb, wc))
    X = np.empty((B, M, N, L), dtype=np.float32)
    for core in range(NCORES):
        b, mh = divmod(core, 2)
        xo = res.results[core]["out"].reshape(MSH, L, N).transpose(0, 2, 1)
        X[b, mh * MSH : (mh + 1) * MSH] = xo
    return X / X.max()


def estimate_ns() -> float:
    """Single-core cost-model estimate of the kernel duration (ns)."""
    from concourse.timeline_sim import TimelineSim

    return TimelineSim(_get_nc()).simulate()
